# revision 41
# baseline (speedup 1.0000x reference)
"""Trainium2 kernel for nn_Group_10: 3x3 replicate-pad conv [4,512,32,32] ->
[4,9728,32,32] (+bias) followed by a per-64-channel-chunk pixel shuffle to
[4,152,256,256].

Sharding: output channels across 8 cores (19 chunks of 64 = 1216 couts each).

v2 design (_build_nc_v2, ~0.75-0.9 ms/iter vs 4.58 ms baseline):
- The pixel shuffle is a pure per-chunk permutation
  out[p, q] = y[cc, h, w], p = (cc>>1)*8 + (w&7), q = (cc&1)*128 + 4h + (w>>3).
  q's top bit is the cout parity (cc&1), so >=1KB DRAM-contiguous store runs
  need *two* couts' data in one SBUF partition.  We get that for free from the
  PE: each PSUM mega-tile [128, 2048] covers 256 couts (4 chunks), partition
  m = g*32 + c holding the cout pair (2c, 2c+1) of chunk 4*t2+g -- the parity
  selects which lhsT (host-side weight gather), not a separate pass.
- Matmuls are N=512 fp32r with a *contiguous* natural-pixel-order moving
  operand ([[WP,16],[1,32]]); the strided shuffled-order AP (32B innermost
  stride) halves the PE stream rate, so the shuffle permutation lives in the
  PSUM->SBUF bias-add copy instead (ACT does parity 0, DVE parity 1).
- After the copy, each partition holds one 8KB DRAM-contiguous run and the
  whole (t2, n) store is ONE DMA with uniform partition stride (128 x 8KB
  descriptors).  The baseline's 512B-run stores (39K descriptors/iter at
  ~97 ns each) were the dominant cost at ~3.8 ms.
- Stores + weight loads issue from the scalar (ACT) HWDGE ring, x loads from
  the sync ring, so store descriptor processing overlaps the matmul stream.
"""

import numpy as np
from contextlib import ExitStack

import concourse.bass as bass
import concourse.mybir as mybir
import concourse.tile as tile
from concourse import bacc
from concourse.bass_utils import run_bass_kernel_spmd

F32 = mybir.dt.float32
F32R = mybir.dt.float32r

N_CORES = 8
B = 4
CIN = 512
H = W_ = 32
COUT = 9728
NCHUNK = COUT // 64            # 152
CH_PER_CORE = NCHUNK // N_CORES  # 19
COUT_CORE = COUT // N_CORES    # 1216
NTILES = 10                    # 1216 padded to 1280 = 10 tiles of 128
HP = WP = 34                   # replicate-padded image
PIX = HP * WP                  # 1156
NCT = CIN // 128               # 4 cin tiles

# within-tile PE output-partition permutation:
#   partition m = chunkbit*64 + parity*32 + cchalf  <->  cout_in_tile =
#   chunkbit*64 + 2*cchalf + parity
_m = np.arange(128)
_chunkbit, _rem = np.divmod(_m, 64)
_parity, _cchalf = np.divmod(_rem, 32)
COUT_IN_TILE = (_chunkbit * 64 + 2 * _cchalf + _parity).astype(np.int64)  # [128]

_nc_cache = None

# ---------------------------------------------------------------------------
# v2: pair-merged PSUM layout.  Each PSUM mega-tile [128, 2048] covers 256
# couts (4 chunks): partition m = g*32 + c holds cout pair (2c, 2c+1) of
# chunk 4*t2+g; free axis = parity*1024 + j*512... (psum order [parity, j,
# qlow]).  MMs stay N=512 contiguous; the bias copy permutes to o_sb order
# [j, parity, qlow]; each partition is then one 8KB-contiguous DRAM run and
# the store is a single DMA per (t2, n) with uniform partition stride.
# ---------------------------------------------------------------------------
NT2 = 5                       # groups of 4 chunks (last has 3)
T2_NPART = [128, 128, 128, 128, 96]


def _build_nc_v2(rep=1, dtype="f32r", skip_mm=False, skip_out=False,
                 skip_copy=False, ct_count=NCT, store_eng="scalar",
                 rhs_nat=True, npair=False, w_eng="scalar"):
    nc = bacc.Bacc("TRN2", target_bir_lowering=False, debug=False,
                   num_devices=N_CORES)

    BF16 = mybir.dt.bfloat16
    MDT = BF16 if dtype == "bf16" else F32
    SDT = BF16 if dtype == "bf16" else F32R
    # "mixw": bf16 stationary weights (FWL-eligible load) x fp32r moving
    WDDT = BF16 if dtype in ("bf16", "mixw") else F32
    WSDT = BF16 if dtype in ("bf16", "mixw") else F32R

    xp = nc.dram_tensor("xp", [B, CIN, HP, WP], MDT, kind="ExternalInput")
    w = nc.dram_tensor("w", [NT2, 128, NCT, 9, 2, 128], WDDT,
                       kind="ExternalInput")
    bias = nc.dram_tensor("bias", [128, NT2 * 2], F32, kind="ExternalInput")
    out = nc.dram_tensor("out", [B, CH_PER_CORE, 256, 256], F32,
                         kind="ExternalOutput")

    with ExitStack() as ctx:
        tc = ctx.enter_context(tile.TileContext(nc))
        xpool = ctx.enter_context(tc.tile_pool(name="xpool", bufs=1))
        wpool = ctx.enter_context(tc.tile_pool(name="wpool", bufs=2))
        opool = ctx.enter_context(tc.tile_pool(name="opool", bufs=3))
        bpool = ctx.enter_context(tc.tile_pool(name="bpool", bufs=1))
        ppool = ctx.enter_context(tc.tile_pool(
            name="ppool", bufs=(1 if npair else 2), space="PSUM"))

        def body():
            x_sb = xpool.tile([128, B * NCT * PIX], SDT)
            xrow = x_sb.ap[0][0]
            xt = x_sb.tensor
            xoff0 = x_sb.offset
            for n in range(B):
                dst = bass.AP(xt, xoff0 + n * NCT * PIX,
                              [[xrow, 128], [PIX, NCT], [1, PIX]])
                src = bass.AP(xp, n * CIN * PIX,
                              [[PIX, 128], [128 * PIX, NCT], [1, PIX]])
                if dtype != "bf16":
                    src = src.bitcast(F32R)
                nc.sync.dma_start(dst, src)

            bias_sb = bpool.tile([128, NT2 * 2], F32)
            nc.sync.dma_start(bias_sb, bias[:])

            def rhs_ap(n, ct, tap, jh):
                dy, dx = divmod(tap, 3)
                if rhs_nat:
                    # natural pixel order: rows [16*jh, 16*jh+16), 32 w contig
                    return bass.AP(
                        xt,
                        xoff0 + (n * NCT + ct) * PIX + (16 * jh + dy) * WP + dx,
                        [[xrow, 128], [WP, 16], [1, 32]],
                    )
                return bass.AP(
                    xt,
                    xoff0 + (n * NCT + ct) * PIX + dy * WP + dx + 4 * jh,
                    [[xrow, 128], [1, 4], [WP, 32], [8, 4]],
                )

            def drain(t2, n, npart, psum):
                o_sb = opool.tile([128, 2048], F32)
                orow = o_sb.ap[0][0]
                prow = psum.ap[0][0]
                for par in range(2):
                    if skip_copy:
                        nc.vector.tensor_scalar_add(
                            o_sb[:, 0:1], psum[:, 0:1],
                            bias_sb[:, t2 * 2:t2 * 2 + 1])
                        break
                    if rhs_nat:
                        # psum holds natural (h, w); shuffle here:
                        # iterate (j, h, r): read h*32 + j + 8r
                        src = bass.AP(psum.tensor,
                                      psum.offset + par * 1024,
                                      [[prow, 128], [1, 8], [32, 32],
                                       [8, 4]])
                    else:
                        src = bass.AP(psum.tensor,
                                      psum.offset + par * 1024,
                                      [[prow, 128], [1, 1024]])
                    dst = bass.AP(o_sb.tensor,
                                  o_sb.offset + par * 128,
                                  [[orow, 128], [256, 8], [1, 128]])
                    eng = nc.scalar if par == 0 else nc.vector
                    if par == 0:
                        eng.add(dst, src, bias_sb[:, t2 * 2:t2 * 2 + 1])
                    else:
                        eng.tensor_scalar_add(
                            dst, src, bias_sb[:, t2 * 2 + 1:t2 * 2 + 2])
                if skip_out:
                    return
                base = (n * CH_PER_CORE + 4 * t2) * 65536
                dstD = bass.AP(out, base, [[2048, npart], [1, 2048]])
                srcS = bass.AP(o_sb.tensor, o_sb.offset,
                               [[orow, npart], [1, 2048]])
                seng = nc.scalar if store_eng == "scalar" else nc.sync
                seng.dma_start(dstD, srcS)

            w_ap = w[:]
            for t2 in range(NT2):
                w_sb = wpool.tile([128, NCT * 9 * 2 * 128], WSDT)
                wsrc = w_ap[t2]
                if dtype not in ("bf16", "mixw"):
                    wsrc = wsrc.bitcast(F32R)
                (nc.scalar if w_eng == "scalar" else nc.sync).dma_start(
                    w_sb, wsrc)
                wrow = w_sb.ap[0][0]
                wt = w_sb.tensor
                woff = w_sb.offset

                def lhsT_ap(ct, tap, par):
                    return bass.AP(
                        wt, woff + ((ct * 9 + tap) * 2 + par) * 128,
                        [[wrow, 128], [1, 128]])

                npart = T2_NPART[t2]
                for n0 in range(0, B, 2 if npair else 1):
                    ns = list(range(n0, n0 + 2)) if npair else [n0]
                    psums = {n: ppool.tile([128, 2048], F32,
                                           name=(f"psum{n % 2}" if npair
                                                 else "psum"))
                             for n in ns}
                    if not skip_mm:
                        for par in range(2):
                            for tap in range(9):
                                for ct in range(ct_count):
                                    for n in ns:
                                        for jh in range(2):
                                            nc.tensor.matmul(
                                                psums[n][
                                                    :, par * 1024 + jh * 512:
                                                    par * 1024 + jh * 512
                                                    + 512],
                                                lhsT_ap(ct, tap, par),
                                                rhs_ap(n, ct, tap, jh),
                                                start=(tap == 0 and ct == 0),
                                                stop=(tap == 8 and
                                                      ct == ct_count - 1),
                                            )
                    else:
                        for n in ns:
                            nc.vector.memset(psums[n][:, 0:1], 0.0)
                    for n in ns:
                        drain(t2, n, npart, psums[n])

        if rep == 1:
            body()
        else:
            with tc.For_i(0, rep):
                body()

    nc.compile()
    return nc


def _host_prep_v2(x, W, b, dtype="f32r"):
    xpad = np.pad(np.asarray(x, dtype=np.float32),
                  ((0, 0), (0, 0), (1, 1), (1, 1)), mode="edge")
    xpad = np.ascontiguousarray(xpad)
    W = np.asarray(W, dtype=np.float32)
    b = np.asarray(b, dtype=np.float32)
    if dtype == "bf16":
        import ml_dtypes
        xpad = xpad.astype(ml_dtypes.bfloat16)

    # cout_local for (t2, m, parity): m = g*32 + c -> (4*t2+g)*64 + 2c + par
    wcast = dtype in ("bf16", "mixw")
    m_ = np.arange(128)
    g_, c_ = np.divmod(m_, 32)

    in_maps = []
    for i in range(N_CORES):
        Ws = W[i * COUT_CORE:(i + 1) * COUT_CORE]          # [1216,512,3,3]
        bs = b[i * COUT_CORE:(i + 1) * COUT_CORE]
        w_dev = np.zeros((NT2, 128, NCT, 9, 2, 128), np.float32)
        bias_dev = np.zeros((128, NT2 * 2), np.float32)
        for t2 in range(NT2):
            npart = T2_NPART[t2]
            for par in range(2):
                cl = (4 * t2 + g_[:npart]) * 64 + 2 * c_[:npart] + par
                Wg = Ws[cl]                                # [npart,512,3,3]
                Wg = Wg.reshape(npart, NCT, 128, 9)        # [m,ct,p,tap]
                w_dev[t2, :, :, :, par, :npart] = Wg.transpose(2, 1, 3, 0)
                bias_dev[:npart, t2 * 2 + par] = bs[cl]
        if wcast:
            import ml_dtypes
            w_dev = w_dev.astype(ml_dtypes.bfloat16)
        else:
            w_dev = np.ascontiguousarray(w_dev)
        in_maps.append({"xp": xpad, "w": w_dev, "bias": bias_dev})
    return in_maps


def _build_nc(rep=1, skip_mm=False, skip_out=False, order="nbk",
              rhs_contig=False, skip_in=False, skip_copy=False,
              ct_count=NCT, dtype="f32r", store_mode="frag"):
    """rep>1 wraps the body in an on-device For_i loop — identical I/O
    signature, used by test.py to measure per-iteration HW time by
    differencing wall-clocks against the rep=1 build.  skip_mm/skip_out
    and order are timing-diagnostic ablations (kernel() uses defaults):
    order="wstat" keeps each weight matrix stationary across (n, bk) by
    holding all four batch PSUM tiles live."""
    nc = bacc.Bacc("TRN2", target_bir_lowering=False, debug=False,
                   num_devices=N_CORES)

    BF16 = mybir.dt.bfloat16
    MDT = BF16 if dtype == "bf16" else F32
    SDT = BF16 if dtype == "bf16" else F32R  # SBUF staging dtype

    xp = nc.dram_tensor("xp", [B, CIN, HP, WP], MDT, kind="ExternalInput")
    w = nc.dram_tensor("w", [NTILES, 128, NCT, 9, 128], MDT,
                       kind="ExternalInput")
    bias = nc.dram_tensor("bias", [128, NTILES], F32, kind="ExternalInput")
    out = nc.dram_tensor("out", [B, CH_PER_CORE, 256, 256], F32,
                         kind="ExternalOutput")

    with ExitStack() as ctx:
        tc = ctx.enter_context(tile.TileContext(nc))
        xpool = ctx.enter_context(tc.tile_pool(name="xpool", bufs=1))
        wpool = ctx.enter_context(tc.tile_pool(name="wpool", bufs=2))
        opool = ctx.enter_context(tc.tile_pool(name="opool", bufs=3))
        bpool = ctx.enter_context(tc.tile_pool(name="bpool", bufs=1))
        mpool = (ctx.enter_context(tc.tile_pool(name="mpool", bufs=3))
                 if store_mode == "repack" else None)
        ppool = ctx.enter_context(tc.tile_pool(
            name="ppool", bufs=(4 if order == "wstat" else 3), space="PSUM"))

        def body():
            # x resident in SBUF: partition = cin%128, free = (n, ct) slabs
            # of 1156 pixels in natural (h, w) padded order.
            x_sb = xpool.tile([128, B * NCT * PIX], SDT)
            xrow = x_sb.ap[0][0]
            xt = x_sb.tensor
            xoff0 = x_sb.offset
            if not skip_in:
                for n in range(B):
                    for ct in range(NCT):
                        dst = bass.AP(xt, xoff0 + (n * NCT + ct) * PIX,
                                      [[xrow, 128], [1, PIX]])
                        src = bass.AP(xp, (n * CIN + ct * 128) * PIX,
                                      [[PIX, 128], [1, PIX]])
                        if dtype != "bf16":
                            src = src.bitcast(F32R)
                        nc.sync.dma_start(dst, src)
            else:
                nc.vector.memset(x_sb[:, 0:1], 0.0)

            bias_sb = bpool.tile([128, NTILES], F32)
            nc.sync.dma_start(bias_sb, bias[:])

            def rhs_ap(n, ct, tap, bk):
                dy, dx = divmod(tap, 3)
                if rhs_contig:
                    # diagnostic: WRONG math, same shape — contiguous stream
                    return bass.AP(
                        xt,
                        xoff0 + (n * NCT + ct) * PIX + dy * WP + dx + 512 * bk,
                        [[xrow, 128], [1, 512]],
                    )
                return bass.AP(
                    xt,
                    xoff0 + (n * NCT + ct) * PIX + dy * WP + dx + 4 * bk,
                    [[xrow, 128], [1, 4], [WP, 32], [8, 4]],
                )

            def store(t, n, psum):
                o_sb = opool.tile([128, 1024], F32)
                if skip_copy:
                    nc.vector.tensor_scalar_add(
                        o_sb[:, 0:1], psum[:, 0:1], bias_sb[:, t:t + 1])
                else:
                    nc.vector.tensor_scalar_add(o_sb, psum, bias_sb[:, t:t + 1])
                if skip_out:
                    return
                orow = o_sb.ap[0][0]
                nchunks = 2 if t < NTILES - 1 else 1
                if store_mode == "repack":
                    # merge parity pairs into one partition (SBUF->SBUF), so
                    # each partition holds one 8KB-contiguous DRAM run
                    npart = nchunks * 32
                    m_sb = mpool.tile([64, 2048], F32)
                    mrow = m_sb.ap[0][0]
                    for cb in range(nchunks):
                        for par in range(2):
                            src = bass.AP(
                                o_sb.tensor,
                                o_sb.offset + (cb * 64 + par * 32) * orow,
                                [[orow, 32], [1, 1024]])
                            dst = bass.AP(
                                m_sb.tensor,
                                m_sb.offset + cb * 32 * mrow + par * 128,
                                [[mrow, 32], [256, 8], [1, 128]])
                            eng = nc.sync if par == 0 else nc.scalar
                            eng.dma_start(dst, src)
                    base = (n * CH_PER_CORE + 2 * t) * 65536
                    dstD = bass.AP(out, base, [[2048, npart], [1, 2048]])
                    srcS = bass.AP(m_sb.tensor, m_sb.offset,
                                   [[mrow, npart], [1, 2048]])
                    nc.sync.dma_start(dstD, srcS)
                    return
                for cb in range(nchunks):
                    src = bass.AP(o_sb.tensor, o_sb.offset + cb * 64 * orow,
                                  [[orow, 64], [128, 8], [1, 128]])
                    base = (n * CH_PER_CORE + 2 * t + cb) * 65536
                    if store_mode == "contig":
                        # diagnostic: WRONG layout, linear dst
                        dst = bass.AP(out, base, [[1024, 64], [1, 1024]])
                    else:
                        dst = bass.AP(out, base,
                                      [[128, 2], [2048, 32], [256, 8],
                                       [1, 128]])
                    nc.sync.dma_start(dst, src)

            w_ap = w[:]
            for t in range(NTILES):
                w_sb = wpool.tile([128, NCT * 9 * 128], SDT)
                if not skip_in:
                    wsrc = w_ap[t]
                    if dtype != "bf16":
                        wsrc = wsrc.bitcast(F32R)
                    nc.sync.dma_start(w_sb, wsrc)
                else:
                    nc.vector.memset(w_sb[:, 0:1], 0.0)
                wrow = w_sb.ap[0][0]
                wt = w_sb.tensor
                woff = w_sb.offset

                def lhsT_ap(ct, tap):
                    return bass.AP(wt, woff + (ct * 9 + tap) * 128,
                                   [[wrow, 128], [1, 128]])

                if order == "wstat":
                    psums = [ppool.tile([128, 1024], F32) for _ in range(B)]
                    if not skip_mm:
                        for tap in range(9):
                            for ct in range(NCT):
                                for n in range(B):
                                    for bk in range(2):
                                        nc.tensor.matmul(
                                            psums[n][:, 512 * bk:
                                                     512 * (bk + 1)],
                                            lhsT_ap(ct, tap),
                                            rhs_ap(n, ct, tap, bk),
                                            start=(tap == 0 and ct == 0),
                                            stop=(tap == 8 and ct == 3),
                                        )
                    for n in range(B):
                        store(t, n, psums[n])
                else:
                    for n in range(B):
                        psum = ppool.tile([128, 1024], F32)
                        if not skip_mm:
                            for bk in range(2):  # PSUM bank = r-halves (w&7)
                                for tap in range(9):
                                    for ct in range(ct_count):
                                        nc.tensor.matmul(
                                            psum[:, 512 * bk:512 * (bk + 1)],
                                            lhsT_ap(ct, tap),
                                            rhs_ap(n, ct, tap, bk),
                                            start=(tap == 0 and ct == 0),
                                            stop=(tap == 8 and
                                                  ct == ct_count - 1),
                                        )
                        else:
                            nc.vector.memset(psum[:, 0:1], 0.0)
                        store(t, n, psum)

        if rep == 1:
            body()
        else:
            with tc.For_i(0, rep):
                body()

    nc.compile()
    return nc


def _host_prep(x, W, b, dtype="f32r"):
    """Build per-core input maps."""
    xpad = np.pad(np.asarray(x, dtype=np.float32),
                  ((0, 0), (0, 0), (1, 1), (1, 1)), mode="edge")
    xpad = np.ascontiguousarray(xpad)
    W = np.asarray(W, dtype=np.float32)
    b = np.asarray(b, dtype=np.float32)
    if dtype == "bf16":
        import ml_dtypes
        xpad = xpad.astype(ml_dtypes.bfloat16)

    in_maps = []
    for i in range(N_CORES):
        Ws = W[i * COUT_CORE:(i + 1) * COUT_CORE]          # [1216,512,3,3]
        Wp = np.zeros((NTILES * 128, CIN, 3, 3), np.float32)
        Wp[:COUT_CORE] = Ws
        gather = (np.arange(NTILES)[:, None] * 128 +
                  COUT_IN_TILE[None, :])                   # [10,128]
        Wg = Wp[gather]                                    # [10,128(m),512,3,3]
        Wg = Wg.reshape(NTILES, 128, NCT, 128, 9)          # [t,m,ct,p,tap]
        w_dev = np.ascontiguousarray(Wg.transpose(0, 3, 2, 4, 1))  # [t,p,ct,tap,m]
        if dtype == "bf16":
            import ml_dtypes
            w_dev = w_dev.astype(ml_dtypes.bfloat16)

        bp = np.zeros((NTILES * 128,), np.float32)
        bp[:COUT_CORE] = b[i * COUT_CORE:(i + 1) * COUT_CORE]
        bias_dev = np.ascontiguousarray(bp[gather].T)      # [128,10]

        in_maps.append({"xp": xpad, "w": w_dev, "bias": bias_dev})
    return in_maps


def _run(in_maps, trace=False):
    global _nc_cache
    if _nc_cache is None:
        _nc_cache = _build_nc_v2()
    return run_bass_kernel_spmd(_nc_cache, in_maps,
                                core_ids=list(range(N_CORES)), trace=trace)


def kernel(x, W, b):
    in_maps = _host_prep_v2(x, W, b)
    res = _run(in_maps)
    outs = [res.results[i]["out"] for i in range(N_CORES)]  # [4,19,256,256]
    full = np.concatenate(outs, axis=1)                     # [4,152,256,256]
    return full



# revision 46
# speedup vs baseline: 1.2368x; 1.2368x over previous
"""Trainium2 kernel for nn_Group_10: 3x3 replicate-pad conv [4,512,32,32] ->
[4,9728,32,32] (+bias) followed by a per-64-channel-chunk pixel shuffle to
[4,152,256,256].

Sharding: output channels across 8 cores (19 chunks of 64 = 1216 couts each).

v2 design (_build_nc_v2, ~0.75-0.9 ms/iter vs 4.58 ms baseline):
- The pixel shuffle is a pure per-chunk permutation
  out[p, q] = y[cc, h, w], p = (cc>>1)*8 + (w&7), q = (cc&1)*128 + 4h + (w>>3).
  q's top bit is the cout parity (cc&1), so >=1KB DRAM-contiguous store runs
  need *two* couts' data in one SBUF partition.  We get that for free from the
  PE: each PSUM mega-tile [128, 2048] covers 256 couts (4 chunks), partition
  m = g*32 + c holding the cout pair (2c, 2c+1) of chunk 4*t2+g -- the parity
  selects which lhsT (host-side weight gather), not a separate pass.
- Matmuls are N=512 fp32r with a *contiguous* natural-pixel-order moving
  operand ([[WP,16],[1,32]]); the strided shuffled-order AP (32B innermost
  stride) halves the PE stream rate, so the shuffle permutation lives in the
  PSUM->SBUF bias-add copy instead (ACT does parity 0, DVE parity 1).
- After the copy, each partition holds one 8KB DRAM-contiguous run and the
  whole (t2, n) store is ONE DMA with uniform partition stride (128 x 8KB
  descriptors).  The baseline's 512B-run stores (39K descriptors/iter at
  ~97 ns each) were the dominant cost at ~3.8 ms.
- Stores + weight loads issue from the scalar (ACT) HWDGE ring, x loads from
  the sync ring, so store descriptor processing overlaps the matmul stream.
- x is staged as four per-batch slabs in a bufs=5 pool so that, under the
  For_i rep loop, the next iteration's slabs prefetch while the current
  iteration's tail matmuls still run (w is bufs=2 and wraps likewise).
"""

import numpy as np
from contextlib import ExitStack

import concourse.bass as bass
import concourse.mybir as mybir
import concourse.tile as tile
from concourse import bacc
from concourse.bass_utils import run_bass_kernel_spmd

F32 = mybir.dt.float32
F32R = mybir.dt.float32r

N_CORES = 8
B = 4
CIN = 512
H = W_ = 32
COUT = 9728
NCHUNK = COUT // 64            # 152
CH_PER_CORE = NCHUNK // N_CORES  # 19
COUT_CORE = COUT // N_CORES    # 1216
NTILES = 10                    # 1216 padded to 1280 = 10 tiles of 128
HP = WP = 34                   # replicate-padded image
PIX = HP * WP                  # 1156
NCT = CIN // 128               # 4 cin tiles

# within-tile PE output-partition permutation:
#   partition m = chunkbit*64 + parity*32 + cchalf  <->  cout_in_tile =
#   chunkbit*64 + 2*cchalf + parity
_m = np.arange(128)
_chunkbit, _rem = np.divmod(_m, 64)
_parity, _cchalf = np.divmod(_rem, 32)
COUT_IN_TILE = (_chunkbit * 64 + 2 * _cchalf + _parity).astype(np.int64)  # [128]

_nc_cache = None

# ---------------------------------------------------------------------------
# v2: pair-merged PSUM layout.  Each PSUM mega-tile [128, 2048] covers 256
# couts (4 chunks): partition m = g*32 + c holds cout pair (2c, 2c+1) of
# chunk 4*t2+g; free axis = parity*1024 + j*512... (psum order [parity, j,
# qlow]).  MMs stay N=512 contiguous; the bias copy permutes to o_sb order
# [j, parity, qlow]; each partition is then one 8KB-contiguous DRAM run and
# the store is a single DMA per (t2, n) with uniform partition stride.
# ---------------------------------------------------------------------------
NT2 = 5                       # groups of 4 chunks (last has 3)
T2_NPART = [128, 128, 128, 128, 96]


def _build_nc_v2(rep=1, dtype="f32r", skip_mm=False, skip_out=False,
                 skip_copy=False, ct_count=NCT, store_eng="scalar",
                 rhs_nat=True, npair=False, w_eng="scalar", xsplit=True):
    nc = bacc.Bacc("TRN2", target_bir_lowering=False, debug=False,
                   num_devices=N_CORES)

    BF16 = mybir.dt.bfloat16
    MDT = BF16 if dtype == "bf16" else F32
    SDT = BF16 if dtype == "bf16" else F32R
    # "mixw": bf16 stationary weights (FWL-eligible load) x fp32r moving
    WDDT = BF16 if dtype in ("bf16", "mixw") else F32
    WSDT = BF16 if dtype in ("bf16", "mixw") else F32R

    xp = nc.dram_tensor("xp", [B, CIN, HP, WP], MDT, kind="ExternalInput")
    w = nc.dram_tensor("w", [NT2, 128, NCT, 9, 2, 128], WDDT,
                       kind="ExternalInput")
    bias = nc.dram_tensor("bias", [128, NT2 * 2], F32, kind="ExternalInput")
    out = nc.dram_tensor("out", [B, CH_PER_CORE, 256, 256], F32,
                         kind="ExternalOutput")

    with ExitStack() as ctx:
        tc = ctx.enter_context(tile.TileContext(nc))
        xpool = ctx.enter_context(tc.tile_pool(
            name="xpool", bufs=(5 if xsplit else 1)))
        wpool = ctx.enter_context(tc.tile_pool(name="wpool", bufs=2))
        opool = ctx.enter_context(tc.tile_pool(name="opool", bufs=3))
        bpool = ctx.enter_context(tc.tile_pool(name="bpool", bufs=1))
        ppool = ctx.enter_context(tc.tile_pool(
            name="ppool", bufs=(1 if npair else 2), space="PSUM"))

        def body():
            # xloc[n] -> (tensor, offset, row_pitch) of batch n's slab.
            # xsplit: one tile per batch in a bufs=5 pool, so the next For_i
            # iteration's slabs prefetch while this iteration still computes.
            xloc = {}
            if xsplit:
                for n in range(B):
                    xs = xpool.tile([128, NCT * PIX], SDT, name="xs")
                    xloc[n] = (xs.tensor, xs.offset, xs.ap[0][0])
            else:
                x_sb = xpool.tile([128, B * NCT * PIX], SDT)
                for n in range(B):
                    xloc[n] = (x_sb.tensor, x_sb.offset + n * NCT * PIX,
                               x_sb.ap[0][0])
            for n in range(B):
                xt_, xo_, xr_ = xloc[n]
                dst = bass.AP(xt_, xo_, [[xr_, 128], [PIX, NCT], [1, PIX]])
                src = bass.AP(xp, n * CIN * PIX,
                              [[PIX, 128], [128 * PIX, NCT], [1, PIX]])
                if dtype != "bf16":
                    src = src.bitcast(F32R)
                nc.sync.dma_start(dst, src)

            bias_sb = bpool.tile([128, NT2 * 2], F32)
            nc.sync.dma_start(bias_sb, bias[:])

            def rhs_ap(n, ct, tap, jh):
                dy, dx = divmod(tap, 3)
                xt_, xo_, xr_ = xloc[n]
                if rhs_nat:
                    # natural pixel order: rows [16*jh, 16*jh+16), 32 w contig
                    return bass.AP(
                        xt_,
                        xo_ + ct * PIX + (16 * jh + dy) * WP + dx,
                        [[xr_, 128], [WP, 16], [1, 32]],
                    )
                return bass.AP(
                    xt_,
                    xo_ + ct * PIX + dy * WP + dx + 4 * jh,
                    [[xr_, 128], [1, 4], [WP, 32], [8, 4]],
                )

            def drain(t2, n, npart, psum):
                o_sb = opool.tile([128, 2048], F32)
                orow = o_sb.ap[0][0]
                prow = psum.ap[0][0]
                for par in range(2):
                    if skip_copy:
                        nc.vector.tensor_scalar_add(
                            o_sb[:, 0:1], psum[:, 0:1],
                            bias_sb[:, t2 * 2:t2 * 2 + 1])
                        break
                    if rhs_nat:
                        # psum holds natural (h, w); shuffle here:
                        # iterate (j, h, r): read h*32 + j + 8r
                        src = bass.AP(psum.tensor,
                                      psum.offset + par * 1024,
                                      [[prow, 128], [1, 8], [32, 32],
                                       [8, 4]])
                    else:
                        src = bass.AP(psum.tensor,
                                      psum.offset + par * 1024,
                                      [[prow, 128], [1, 1024]])
                    dst = bass.AP(o_sb.tensor,
                                  o_sb.offset + par * 128,
                                  [[orow, 128], [256, 8], [1, 128]])
                    eng = nc.scalar if par == 0 else nc.vector
                    if par == 0:
                        eng.add(dst, src, bias_sb[:, t2 * 2:t2 * 2 + 1])
                    else:
                        eng.tensor_scalar_add(
                            dst, src, bias_sb[:, t2 * 2 + 1:t2 * 2 + 2])
                if skip_out:
                    return
                base = (n * CH_PER_CORE + 4 * t2) * 65536
                dstD = bass.AP(out, base, [[2048, npart], [1, 2048]])
                srcS = bass.AP(o_sb.tensor, o_sb.offset,
                               [[orow, npart], [1, 2048]])
                seng = nc.scalar if store_eng == "scalar" else nc.sync
                seng.dma_start(dstD, srcS)

            w_ap = w[:]
            for t2 in range(NT2):
                w_sb = wpool.tile([128, NCT * 9 * 2 * 128], WSDT)
                wsrc = w_ap[t2]
                if dtype not in ("bf16", "mixw"):
                    wsrc = wsrc.bitcast(F32R)
                (nc.scalar if w_eng == "scalar" else nc.sync).dma_start(
                    w_sb, wsrc)
                wrow = w_sb.ap[0][0]
                wt = w_sb.tensor
                woff = w_sb.offset

                def lhsT_ap(ct, tap, par):
                    return bass.AP(
                        wt, woff + ((ct * 9 + tap) * 2 + par) * 128,
                        [[wrow, 128], [1, 128]])

                npart = T2_NPART[t2]
                for n0 in range(0, B, 2 if npair else 1):
                    ns = list(range(n0, n0 + 2)) if npair else [n0]
                    psums = {n: ppool.tile([128, 2048], F32,
                                           name=(f"psum{n % 2}" if npair
                                                 else "psum"))
                             for n in ns}
                    if not skip_mm:
                        for par in range(2):
                            for tap in range(9):
                                for ct in range(ct_count):
                                    for n in ns:
                                        for jh in range(2):
                                            nc.tensor.matmul(
                                                psums[n][
                                                    :, par * 1024 + jh * 512:
                                                    par * 1024 + jh * 512
                                                    + 512],
                                                lhsT_ap(ct, tap, par),
                                                rhs_ap(n, ct, tap, jh),
                                                start=(tap == 0 and ct == 0),
                                                stop=(tap == 8 and
                                                      ct == ct_count - 1),
                                            )
                    else:
                        for n in ns:
                            nc.vector.memset(psums[n][:, 0:1], 0.0)
                    for n in ns:
                        drain(t2, n, npart, psums[n])

        if rep == 1:
            body()
        else:
            with tc.For_i(0, rep):
                body()

    nc.compile()
    return nc


def _host_prep_v2(x, W, b, dtype="f32r"):
    xpad = np.pad(np.asarray(x, dtype=np.float32),
                  ((0, 0), (0, 0), (1, 1), (1, 1)), mode="edge")
    xpad = np.ascontiguousarray(xpad)
    W = np.asarray(W, dtype=np.float32)
    b = np.asarray(b, dtype=np.float32)
    if dtype == "bf16":
        import ml_dtypes
        xpad = xpad.astype(ml_dtypes.bfloat16)

    # cout_local for (t2, m, parity): m = g*32 + c -> (4*t2+g)*64 + 2c + par
    wcast = dtype in ("bf16", "mixw")
    m_ = np.arange(128)
    g_, c_ = np.divmod(m_, 32)

    in_maps = []
    for i in range(N_CORES):
        Ws = W[i * COUT_CORE:(i + 1) * COUT_CORE]          # [1216,512,3,3]
        bs = b[i * COUT_CORE:(i + 1) * COUT_CORE]
        w_dev = np.zeros((NT2, 128, NCT, 9, 2, 128), np.float32)
        bias_dev = np.zeros((128, NT2 * 2), np.float32)
        for t2 in range(NT2):
            npart = T2_NPART[t2]
            for par in range(2):
                cl = (4 * t2 + g_[:npart]) * 64 + 2 * c_[:npart] + par
                Wg = Ws[cl]                                # [npart,512,3,3]
                Wg = Wg.reshape(npart, NCT, 128, 9)        # [m,ct,p,tap]
                w_dev[t2, :, :, :, par, :npart] = Wg.transpose(2, 1, 3, 0)
                bias_dev[:npart, t2 * 2 + par] = bs[cl]
        if wcast:
            import ml_dtypes
            w_dev = w_dev.astype(ml_dtypes.bfloat16)
        else:
            w_dev = np.ascontiguousarray(w_dev)
        in_maps.append({"xp": xpad, "w": w_dev, "bias": bias_dev})
    return in_maps


def _build_nc(rep=1, skip_mm=False, skip_out=False, order="nbk",
              rhs_contig=False, skip_in=False, skip_copy=False,
              ct_count=NCT, dtype="f32r", store_mode="frag"):
    """rep>1 wraps the body in an on-device For_i loop — identical I/O
    signature, used by test.py to measure per-iteration HW time by
    differencing wall-clocks against the rep=1 build.  skip_mm/skip_out
    and order are timing-diagnostic ablations (kernel() uses defaults):
    order="wstat" keeps each weight matrix stationary across (n, bk) by
    holding all four batch PSUM tiles live."""
    nc = bacc.Bacc("TRN2", target_bir_lowering=False, debug=False,
                   num_devices=N_CORES)

    BF16 = mybir.dt.bfloat16
    MDT = BF16 if dtype == "bf16" else F32
    SDT = BF16 if dtype == "bf16" else F32R  # SBUF staging dtype

    xp = nc.dram_tensor("xp", [B, CIN, HP, WP], MDT, kind="ExternalInput")
    w = nc.dram_tensor("w", [NTILES, 128, NCT, 9, 128], MDT,
                       kind="ExternalInput")
    bias = nc.dram_tensor("bias", [128, NTILES], F32, kind="ExternalInput")
    out = nc.dram_tensor("out", [B, CH_PER_CORE, 256, 256], F32,
                         kind="ExternalOutput")

    with ExitStack() as ctx:
        tc = ctx.enter_context(tile.TileContext(nc))
        xpool = ctx.enter_context(tc.tile_pool(name="xpool", bufs=1))
        wpool = ctx.enter_context(tc.tile_pool(name="wpool", bufs=2))
        opool = ctx.enter_context(tc.tile_pool(name="opool", bufs=3))
        bpool = ctx.enter_context(tc.tile_pool(name="bpool", bufs=1))
        mpool = (ctx.enter_context(tc.tile_pool(name="mpool", bufs=3))
                 if store_mode == "repack" else None)
        ppool = ctx.enter_context(tc.tile_pool(
            name="ppool", bufs=(4 if order == "wstat" else 3), space="PSUM"))

        def body():
            # x resident in SBUF: partition = cin%128, free = (n, ct) slabs
            # of 1156 pixels in natural (h, w) padded order.
            x_sb = xpool.tile([128, B * NCT * PIX], SDT)
            xrow = x_sb.ap[0][0]
            xt = x_sb.tensor
            xoff0 = x_sb.offset
            if not skip_in:
                for n in range(B):
                    for ct in range(NCT):
                        dst = bass.AP(xt, xoff0 + (n * NCT + ct) * PIX,
                                      [[xrow, 128], [1, PIX]])
                        src = bass.AP(xp, (n * CIN + ct * 128) * PIX,
                                      [[PIX, 128], [1, PIX]])
                        if dtype != "bf16":
                            src = src.bitcast(F32R)
                        nc.sync.dma_start(dst, src)
            else:
                nc.vector.memset(x_sb[:, 0:1], 0.0)

            bias_sb = bpool.tile([128, NTILES], F32)
            nc.sync.dma_start(bias_sb, bias[:])

            def rhs_ap(n, ct, tap, bk):
                dy, dx = divmod(tap, 3)
                if rhs_contig:
                    # diagnostic: WRONG math, same shape — contiguous stream
                    return bass.AP(
                        xt,
                        xoff0 + (n * NCT + ct) * PIX + dy * WP + dx + 512 * bk,
                        [[xrow, 128], [1, 512]],
                    )
                return bass.AP(
                    xt,
                    xoff0 + (n * NCT + ct) * PIX + dy * WP + dx + 4 * bk,
                    [[xrow, 128], [1, 4], [WP, 32], [8, 4]],
                )

            def store(t, n, psum):
                o_sb = opool.tile([128, 1024], F32)
                if skip_copy:
                    nc.vector.tensor_scalar_add(
                        o_sb[:, 0:1], psum[:, 0:1], bias_sb[:, t:t + 1])
                else:
                    nc.vector.tensor_scalar_add(o_sb, psum, bias_sb[:, t:t + 1])
                if skip_out:
                    return
                orow = o_sb.ap[0][0]
                nchunks = 2 if t < NTILES - 1 else 1
                if store_mode == "repack":
                    # merge parity pairs into one partition (SBUF->SBUF), so
                    # each partition holds one 8KB-contiguous DRAM run
                    npart = nchunks * 32
                    m_sb = mpool.tile([64, 2048], F32)
                    mrow = m_sb.ap[0][0]
                    for cb in range(nchunks):
                        for par in range(2):
                            src = bass.AP(
                                o_sb.tensor,
                                o_sb.offset + (cb * 64 + par * 32) * orow,
                                [[orow, 32], [1, 1024]])
                            dst = bass.AP(
                                m_sb.tensor,
                                m_sb.offset + cb * 32 * mrow + par * 128,
                                [[mrow, 32], [256, 8], [1, 128]])
                            eng = nc.sync if par == 0 else nc.scalar
                            eng.dma_start(dst, src)
                    base = (n * CH_PER_CORE + 2 * t) * 65536
                    dstD = bass.AP(out, base, [[2048, npart], [1, 2048]])
                    srcS = bass.AP(m_sb.tensor, m_sb.offset,
                                   [[mrow, npart], [1, 2048]])
                    nc.sync.dma_start(dstD, srcS)
                    return
                for cb in range(nchunks):
                    src = bass.AP(o_sb.tensor, o_sb.offset + cb * 64 * orow,
                                  [[orow, 64], [128, 8], [1, 128]])
                    base = (n * CH_PER_CORE + 2 * t + cb) * 65536
                    if store_mode == "contig":
                        # diagnostic: WRONG layout, linear dst
                        dst = bass.AP(out, base, [[1024, 64], [1, 1024]])
                    else:
                        dst = bass.AP(out, base,
                                      [[128, 2], [2048, 32], [256, 8],
                                       [1, 128]])
                    nc.sync.dma_start(dst, src)

            w_ap = w[:]
            for t in range(NTILES):
                w_sb = wpool.tile([128, NCT * 9 * 128], SDT)
                if not skip_in:
                    wsrc = w_ap[t]
                    if dtype != "bf16":
                        wsrc = wsrc.bitcast(F32R)
                    nc.sync.dma_start(w_sb, wsrc)
                else:
                    nc.vector.memset(w_sb[:, 0:1], 0.0)
                wrow = w_sb.ap[0][0]
                wt = w_sb.tensor
                woff = w_sb.offset

                def lhsT_ap(ct, tap):
                    return bass.AP(wt, woff + (ct * 9 + tap) * 128,
                                   [[wrow, 128], [1, 128]])

                if order == "wstat":
                    psums = [ppool.tile([128, 1024], F32) for _ in range(B)]
                    if not skip_mm:
                        for tap in range(9):
                            for ct in range(NCT):
                                for n in range(B):
                                    for bk in range(2):
                                        nc.tensor.matmul(
                                            psums[n][:, 512 * bk:
                                                     512 * (bk + 1)],
                                            lhsT_ap(ct, tap),
                                            rhs_ap(n, ct, tap, bk),
                                            start=(tap == 0 and ct == 0),
                                            stop=(tap == 8 and ct == 3),
                                        )
                    for n in range(B):
                        store(t, n, psums[n])
                else:
                    for n in range(B):
                        psum = ppool.tile([128, 1024], F32)
                        if not skip_mm:
                            for bk in range(2):  # PSUM bank = r-halves (w&7)
                                for tap in range(9):
                                    for ct in range(ct_count):
                                        nc.tensor.matmul(
                                            psum[:, 512 * bk:512 * (bk + 1)],
                                            lhsT_ap(ct, tap),
                                            rhs_ap(n, ct, tap, bk),
                                            start=(tap == 0 and ct == 0),
                                            stop=(tap == 8 and
                                                  ct == ct_count - 1),
                                        )
                        else:
                            nc.vector.memset(psum[:, 0:1], 0.0)
                        store(t, n, psum)

        if rep == 1:
            body()
        else:
            with tc.For_i(0, rep):
                body()

    nc.compile()
    return nc


def _host_prep(x, W, b, dtype="f32r"):
    """Build per-core input maps."""
    xpad = np.pad(np.asarray(x, dtype=np.float32),
                  ((0, 0), (0, 0), (1, 1), (1, 1)), mode="edge")
    xpad = np.ascontiguousarray(xpad)
    W = np.asarray(W, dtype=np.float32)
    b = np.asarray(b, dtype=np.float32)
    if dtype == "bf16":
        import ml_dtypes
        xpad = xpad.astype(ml_dtypes.bfloat16)

    in_maps = []
    for i in range(N_CORES):
        Ws = W[i * COUT_CORE:(i + 1) * COUT_CORE]          # [1216,512,3,3]
        Wp = np.zeros((NTILES * 128, CIN, 3, 3), np.float32)
        Wp[:COUT_CORE] = Ws
        gather = (np.arange(NTILES)[:, None] * 128 +
                  COUT_IN_TILE[None, :])                   # [10,128]
        Wg = Wp[gather]                                    # [10,128(m),512,3,3]
        Wg = Wg.reshape(NTILES, 128, NCT, 128, 9)          # [t,m,ct,p,tap]
        w_dev = np.ascontiguousarray(Wg.transpose(0, 3, 2, 4, 1))  # [t,p,ct,tap,m]
        if dtype == "bf16":
            import ml_dtypes
            w_dev = w_dev.astype(ml_dtypes.bfloat16)

        bp = np.zeros((NTILES * 128,), np.float32)
        bp[:COUT_CORE] = b[i * COUT_CORE:(i + 1) * COUT_CORE]
        bias_dev = np.ascontiguousarray(bp[gather].T)      # [128,10]

        in_maps.append({"xp": xpad, "w": w_dev, "bias": bias_dev})
    return in_maps


def _run(in_maps, trace=False):
    global _nc_cache
    if _nc_cache is None:
        _nc_cache = _build_nc_v2()
    return run_bass_kernel_spmd(_nc_cache, in_maps,
                                core_ids=list(range(N_CORES)), trace=trace)


def kernel(x, W, b):
    in_maps = _host_prep_v2(x, W, b)
    res = _run(in_maps)
    outs = [res.results[i]["out"] for i in range(N_CORES)]  # [4,19,256,256]
    full = np.concatenate(outs, axis=1)                     # [4,152,256,256]
    return full

